# revision 3
# baseline (speedup 1.0000x reference)
import numpy as np
import jax
import jax.numpy as jnp

# Problem constants (nn_AttentionDecoder2D): hardcoded per contract.
B, T, VOCAB, H, C, HS = 128, 32, 32000, 512, 512, 7
HW = HS * HS
NCORES = 8
BL = B // NCORES  # per-core batch shard


def _decode(visual, ew_seq, h0, c0, att_Wv, att_Wh, att_wo,
            U_g, Z_g, b_g, out_W1, out_b1, out_W2, out_b2):
    # Per-shard: visual [BL,C,HS,HS], ew_seq [T,BL,4H] (= emb[tok] @ W_g, host-precomputed)
    feats = visual.reshape(BL, C, HW)
    Vproj = jnp.einsum('bcn,hc->bnh', feats, att_Wv)      # [BL,HW,H]

    def step(carry, ew):
        h, c = carry
        Hp = h @ att_Wh.T                                  # [BL,H]
        scores = jnp.tanh(Vproj + Hp[:, None, :]) @ att_wo  # [BL,HW]
        alpha = jax.nn.softmax(scores, axis=1)
        attended = jnp.einsum('bcn,bn->bc', feats, alpha)   # [BL,C]
        gates = ew + h @ U_g + attended @ Z_g + b_g          # [BL,4H]
        i, f, g, o = jnp.split(gates, 4, axis=1)
        i, f, o = jax.nn.sigmoid(i), jax.nn.sigmoid(f), jax.nn.sigmoid(o)
        c = f * c + i * jnp.tanh(g)
        h = o * jnp.tanh(c)
        hid = jnp.concatenate([attended, h], axis=1)        # [BL,C+H]
        return (h, c), hid

    (_, _), hids = jax.lax.scan(step, (h0, c0), ew_seq)     # [T,BL,C+H]

    # Vocab head batched over all timesteps; output stays scan-major [T,BL,V]
    x = hids.reshape(T * BL, C + H)
    y = jax.nn.relu(x @ out_W1.T + out_b1) @ out_W2.T + out_b2
    return y.reshape(T, BL, VOCAB)


_pmapped = jax.pmap(
    _decode,
    in_axes=(0, 1, 0, 0) + (None,) * 10,
    out_axes=0,
)


def _stage(inputs):
    emb = np.asarray(inputs['emb'], np.float32)
    W_g = np.asarray(inputs['W_g'], np.float32)
    cap = np.asarray(inputs['caption_inputs']).astype(np.int64)  # [B,T]
    # Host-side embedding gather + token-gate precompute: EW[t,b] = emb[cap[b,t]] @ W_g
    E = emb[cap]                                            # [B,T,H]
    EW = E.reshape(B * T, H) @ W_g                          # [B*T,4H]
    EW = EW.reshape(B, T, 4 * H).transpose(1, 0, 2)         # [T,B,4H]
    EW = np.ascontiguousarray(EW.reshape(T, NCORES, BL, 4 * H))

    vis = np.asarray(inputs['visual_inputs'], np.float32).reshape(NCORES, BL, C, HS, HS)
    h0s = np.asarray(inputs['h0'], np.float32).reshape(NCORES, BL, H)
    c0s = np.asarray(inputs['c0'], np.float32).reshape(NCORES, BL, H)
    f32 = lambda k: jnp.asarray(inputs[k], jnp.float32)
    return (vis, EW, h0s, c0s,
            f32('att_Wv'), f32('att_Wh'), f32('att_wo'),
            f32('U_g'), f32('Z_g'), f32('b_g'),
            f32('out_W1'), f32('out_b1'), f32('out_W2'), f32('out_b2'))


def kernel(**inputs):
    out = _pmapped(*_stage(inputs))
    out = np.asarray(out)                                   # [NCORES,T,BL,V]
    return np.ascontiguousarray(out.transpose(0, 2, 1, 3)).reshape(B, T, VOCAB)


# revision 5
# speedup vs baseline: 5.9246x; 5.9246x over previous
import numpy as np
import jax
import jax.numpy as jnp

# Problem constants (nn_AttentionDecoder2D): hardcoded per contract.
B, T, VOCAB, H, C, HS = 128, 32, 32000, 512, 512, 7
HW = HS * HS
NCORES = 8
BL = B // NCORES  # per-core batch shard


def _decode(visual, ew_seq, h0, c0, att_Wv, att_Wh, att_wo,
            U_g, Z_g, b_g, out_W1, out_b1, out_W2, out_b2):
    # Per-shard: visual [BL,C,HS,HS], ew_seq [T,BL,4H] (= emb[tok] @ W_g, host-precomputed)
    feats = visual.reshape(BL, C, HW)
    Vproj = jnp.einsum('bcn,hc->bnh', feats, att_Wv)      # [BL,HW,H]

    def step(carry, ew):
        h, c = carry
        Hp = h @ att_Wh.T                                  # [BL,H]
        scores = jnp.tanh(Vproj + Hp[:, None, :]) @ att_wo  # [BL,HW]
        alpha = jax.nn.softmax(scores, axis=1)
        attended = jnp.einsum('bcn,bn->bc', feats, alpha)   # [BL,C]
        gates = ew + h @ U_g + attended @ Z_g + b_g          # [BL,4H]
        i, f, g, o = jnp.split(gates, 4, axis=1)
        i, f, o = jax.nn.sigmoid(i), jax.nn.sigmoid(f), jax.nn.sigmoid(o)
        c = f * c + i * jnp.tanh(g)
        h = o * jnp.tanh(c)
        hid = jnp.concatenate([attended, h], axis=1)        # [BL,C+H]
        return (h, c), hid

    (_, _), hids = jax.lax.scan(step, (h0, c0), ew_seq)     # [T,BL,C+H]

    # Vocab head batched over all timesteps; output stays scan-major [T,BL,V]
    x = hids.reshape(T * BL, C + H)
    y = jax.nn.relu(x @ out_W1.T + out_b1) @ out_W2.T + out_b2
    return y.reshape(T, BL, VOCAB)


_pmapped = jax.pmap(_decode, in_axes=0, out_axes=0)


def _stage(inputs):
    emb = np.asarray(inputs['emb'], np.float32)
    W_g = np.asarray(inputs['W_g'], np.float32)
    cap = np.asarray(inputs['caption_inputs']).astype(np.int64)  # [B,T]
    # Host-side embedding gather + token-gate precompute: EW[t,b] = emb[cap[b,t]] @ W_g
    E = emb[cap]                                            # [B,T,H]
    EW = E.reshape(B * T, H) @ W_g                          # [B*T,4H]
    EW = EW.reshape(B, T, 4 * H).transpose(1, 0, 2)         # [T,B,4H]
    # [NCORES, T, BL, 4H]: shard axis leading
    EW = np.ascontiguousarray(EW.reshape(T, NCORES, BL, 4 * H).transpose(1, 0, 2, 3))

    vis = np.asarray(inputs['visual_inputs'], np.float32).reshape(NCORES, BL, C, HS, HS)
    h0s = np.asarray(inputs['h0'], np.float32).reshape(NCORES, BL, H)
    c0s = np.asarray(inputs['c0'], np.float32).reshape(NCORES, BL, H)

    devs = jax.local_devices()[:NCORES]
    shard = lambda a: jax.device_put_sharded(list(a), devs)
    repl = lambda k: jax.device_put_replicated(
        jnp.asarray(np.asarray(inputs[k], np.float32)), devs)
    return (shard(vis), shard(EW), shard(h0s), shard(c0s),
            repl('att_Wv'), repl('att_Wh'), repl('att_wo'),
            repl('U_g'), repl('Z_g'), repl('b_g'),
            repl('out_W1'), repl('out_b1'), repl('out_W2'), repl('out_b2'))


def kernel(**inputs):
    out = _pmapped(*_stage(inputs))
    out = np.asarray(out)                                   # [NCORES,T,BL,V]
    return np.ascontiguousarray(out.transpose(0, 2, 1, 3)).reshape(B, T, VOCAB)


# revision 9
# speedup vs baseline: 8.1883x; 1.3821x over previous
import numpy as np
import jax
import jax.numpy as jnp

# Problem constants (nn_AttentionDecoder2D): hardcoded per contract.
B, T, VOCAB, H, C, HS = 128, 32, 32000, 512, 512, 7
HW = HS * HS
NCORES = 8
BL = B // NCORES  # per-core batch shard


def _decode(visual, e_seq, h0, c0, att_Wv, att_Wh, att_wo,
            W_g, U_g, Z_g, b_g, out_W1, out_b1, out_W2, out_b2):
    # Per-shard: visual [BL,C,HS,HS], e_seq [T,BL,H] (= emb[tok], host-gathered)
    feats = visual.reshape(BL, C, HW)
    Vproj = jnp.einsum('bcn,hc->bnh', feats, att_Wv)      # [BL,HW,H]
    ew_seq = jnp.einsum('tbh,hg->tbg', e_seq, W_g)         # [T,BL,4H]

    def step(carry, ew):
        h, c = carry
        Hp = h @ att_Wh.T                                  # [BL,H]
        scores = jnp.tanh(Vproj + Hp[:, None, :]) @ att_wo  # [BL,HW]
        alpha = jax.nn.softmax(scores, axis=1)
        attended = jnp.einsum('bcn,bn->bc', feats, alpha)   # [BL,C]
        gates = ew + h @ U_g + attended @ Z_g + b_g          # [BL,4H]
        i, f, g, o = jnp.split(gates, 4, axis=1)
        i, f, o = jax.nn.sigmoid(i), jax.nn.sigmoid(f), jax.nn.sigmoid(o)
        c = f * c + i * jnp.tanh(g)
        h = o * jnp.tanh(c)
        hid = jnp.concatenate([attended, h], axis=1)        # [BL,C+H]
        return (h, c), hid

    (_, _), hids = jax.lax.scan(step, (h0, c0), ew_seq, unroll=T)  # [T,BL,C+H]

    # Vocab head batched over all timesteps; output stays scan-major [T,BL,V]
    x = hids.reshape(T * BL, C + H)
    y = jax.nn.relu(x @ out_W1.T + out_b1) @ out_W2.T + out_b2
    return y.reshape(T, BL, VOCAB)


_pmapped = jax.pmap(_decode, in_axes=0, out_axes=0)


def _stage(inputs):
    emb = np.asarray(inputs['emb'], np.float32)
    W_g = np.asarray(inputs['W_g'], np.float32)
    cap = np.asarray(inputs['caption_inputs']).astype(np.int64)  # [B,T]
    # Host-side embedding gather (device gather miscompiles on this backend)
    E = emb[cap]                                            # [B,T,H]
    E = E.transpose(1, 0, 2)                                # [T,B,H]
    # [NCORES, T, BL, H]: shard axis leading
    E = np.ascontiguousarray(E.reshape(T, NCORES, BL, H).transpose(1, 0, 2, 3))

    vis = np.asarray(inputs['visual_inputs'], np.float32).reshape(NCORES, BL, C, HS, HS)
    h0s = np.asarray(inputs['h0'], np.float32).reshape(NCORES, BL, H)
    c0s = np.asarray(inputs['c0'], np.float32).reshape(NCORES, BL, H)

    devs = jax.local_devices()[:NCORES]
    shard = lambda a: jax.device_put_sharded(list(a), devs)
    repl = lambda k: jax.device_put_replicated(
        jnp.asarray(np.asarray(inputs[k], np.float32)), devs)
    return (shard(vis), shard(E), shard(h0s), shard(c0s),
            repl('att_Wv'), repl('att_Wh'), repl('att_wo'),
            repl('W_g'), repl('U_g'), repl('Z_g'), repl('b_g'),
            repl('out_W1'), repl('out_b1'), repl('out_W2'), repl('out_b2'))


def kernel(**inputs):
    out = _pmapped(*_stage(inputs))
    out = np.asarray(out)                                   # [NCORES,T,BL,V]
    return np.ascontiguousarray(out.transpose(0, 2, 1, 3)).reshape(B, T, VOCAB)
